# revision 5
# baseline (speedup 1.0000x reference)
"""Trainium2 Bass kernel for the butterfly-CNN problem (nn_CNNLayer_30296699306356).

Network (see problem reference): input conv (k=2,s=2, 1->8 ch) + 10 butterfly
conv levels (k=2,s=2, channels double each level, relu, zero biases) + a
per-block dense matmul (1024 blocks of [8,2]) at the end.

Strategy (memory-regime; weights are ~358 MB fp32 dominated by levels 8-10):
  - Levels 5..9 run in bf16 (weights + activations, fp32 PSUM accumulation).
    Level 10 weights are float8e3 (e3m4) with per-output-channel scales that
    are folded into fea_dense on the host (relu commutes with positive
    scales), halving the dominant weight stream. Measured rel err ~1.6e-2
    (gate 2e-2, deterministic inputs).
  - Levels in..8 are replicated on all 8 cores. Levels in..4 use an
    "im2col-packed" layout: activations are stored as [128 partitions =
    (wsub, ch), wHi, b] so every matmul is a full 128-wide contraction with a
    single block-structured 128x128 stationary weight.
  - Levels 9 and 10 shard the OUTPUT channels across the 8 cores. One 8-core
    AllGather reassembles x9. Level 10's output channels align exactly with
    the fea_dense block shard, so no gather is needed after level 10.
  - w9 and w10 are held fully RESIDENT in SBUF (8 + 16 slice DMAs into
    dedicated tiles) so the weight stream runs as one continuous FIFO and
    never stalls behind the collective window.
  - Level 10 runs "orientation B" (activations stationary, weights moving).
  - Final block einsum is done on the Vector engine (mult + grouped reduce).

kernel(**inputs) takes the FULL unsharded inputs and returns the FULL output.
"""

import ml_dtypes
import numpy as np

NCORES = 8
B = 16
P = 128
C = 8
NLVL = 10
BF16 = ml_dtypes.bfloat16
FP8 = ml_dtypes.float8_e3m4
FP8_SCALE_TARGET = 14.0

_CACHE = {}


# ---------------------------------------------------------------- host prep

def _host_prep(inputs):
    """Build the per-core input maps (numpy only)."""
    ind = np.ascontiguousarray(np.asarray(inputs["in_data"], np.float32))
    f = {l: np.asarray(inputs[f"f{l}"], np.float32) for l in range(1, NLVL + 1)}
    f0 = np.asarray(inputs["in_filter"], np.float32)     # [2, 1, 8]
    fd = np.asarray(inputs["fea_dense"], np.float32)     # [1024, 8, 2]

    shared = {}
    # r0 [32, 64, 16]: r0[row, wHi, b] = in[b, wHi*32 + row]
    shared["r0"] = np.ascontiguousarray(
        ind[:, :, 0].reshape(B, 64, 32).transpose(2, 1, 0))

    # w0 [32, 128]: rows (2*wsub + k), cols (wsub*8 + co)
    w0 = np.zeros((32, 128), np.float32)
    for wsub in range(16):
        for k in range(2):
            w0[2 * wsub + k, wsub * 8:wsub * 8 + 8] = f0[k, 0, :]
    shared["w0"] = w0

    # packed levels 1..4 stacked: wpk [4, 128, 128]
    wpk = np.zeros((4, 128, 128), np.float32)
    for lvl in range(1, 5):
        cin = 2 ** (lvl - 1) * C
        cout = 2 ** lvl * C
        s_out = (128 // cin) // 2
        for wso in range(s_out):
            for k in range(2):
                wsi = 2 * wso + k
                wpk[lvl - 1, wsi * cin:(wsi + 1) * cin,
                    wso * cout:(wso + 1) * cout] = f[lvl][k]
    shared["wpk"] = wpk

    # w5/w6/w7 mega-packed [128, 10752] bf16 (kt-major per level), one tile
    w5v = f[5].astype(BF16).reshape(2, 1, 128, 256)
    w6v = f[6].astype(BF16).reshape(2, 2, 128, 512)
    w7v = f[7].astype(BF16).reshape(2, 4, 128, 1024)
    shared["wmid"] = np.ascontiguousarray(np.concatenate([
        w5v.transpose(2, 0, 1, 3).reshape(128, 512),
        w6v.transpose(2, 0, 1, 3).reshape(128, 2048),
        w7v.transpose(2, 0, 1, 3).reshape(128, 8192)], axis=1))

    # f8 is REPLICATED: co-major chunks [4, 128, kt=16, co=512], kt = k*8 + cit
    f8b = f[8].astype(BF16)
    w8full = np.stack([
        np.ascontiguousarray(
            f8b[:, :, c * 512:(c + 1) * 512]
            .reshape(2, 8, 128, 512).transpose(2, 0, 1, 3).reshape(128, 16, 512))
        for c in range(4)])
    shared["w8"] = w8full

    # f9 output-channel shards, packed into 4-ci-tile chunks:
    # [8, 128, 4, 512]; chunk m = k*4 + q, cit = q*4+j
    w9s = []
    f9b = f[9].astype(BF16)
    for r in range(NCORES):
        blk = f9b[:, :, r * 512:(r + 1) * 512]
        v = blk.reshape(2, 4, 4, 128, 512).transpose(0, 1, 3, 2, 4)
        w9s.append(np.ascontiguousarray(v.reshape(8, 128, 4, 512)))

    # f10 output-channel shards in float8_e3m4 with per-output-channel scales
    # (folded into fea_dense below): [16, 128, 4, 1024] fp8,
    # chunk m = k*8 + q, cit = q*4 + j
    s10 = np.max(np.abs(f[10]), axis=(0, 1)) / FP8_SCALE_TARGET  # [8192]
    f10q = (f[10] / s10[None, None, :]).astype(FP8)
    w10s = []
    for r in range(NCORES):
        blk = f10q[:, :, r * 1024:(r + 1) * 1024]
        v = blk.reshape(2, 8, 4, 128, 1024).transpose(0, 1, 3, 2, 4)
        w10s.append(np.ascontiguousarray(v.reshape(16, 128, 4, 1024)))

    # fea_dense shard with the fp8 scales folded in, per-o flattened, tiled
    # over the 16 batch partitions
    fds = []
    for r in range(NCORES):
        blk = fd[r * 128:(r + 1) * 128]                    # [128, 8, 2]
        flat = blk.transpose(2, 0, 1).reshape(2, 1024)     # [o, 1024]
        flat = flat * s10[r * 1024:(r + 1) * 1024][None, :]
        fds.append(np.ascontiguousarray(
            np.broadcast_to(flat[None], (B, 2, 1024)).astype(np.float32)))

    in_maps = []
    for r in range(NCORES):
        m = dict(shared)
        m["w9"] = w9s[r]
        m["w10"] = w10s[r]
        m["fdt"] = fds[r]
        in_maps.append(m)
    return in_maps


# ---------------------------------------------------------------- bass build

def _build():
    import concourse.bass as bass
    import concourse.mybir as mybir
    import concourse.tile as tile
    from concourse import bacc

    f32 = mybir.dt.float32
    bf16 = mybir.dt.bfloat16
    fp8 = mybir.dt.float8e3
    RELU = mybir.ActivationFunctionType.Relu

    nc = bacc.Bacc("TRN2", target_bir_lowering=False, debug=False,
                   num_devices=NCORES)

    def inp(name, shape, dt=f32):
        return nc.dram_tensor(name, shape, dt, kind="ExternalInput").ap()

    r0 = inp("r0", [32, 64, 16])
    w0 = inp("w0", [32, 128])
    wpk = inp("wpk", [4, 128, 128])
    wmid = inp("wmid", [128, 10752], bf16)
    w8 = inp("w8", [4, 128, 16, 512], bf16)
    w9 = inp("w9", [8, 128, 4, 512], bf16)
    w10 = inp("w10", [16, 128, 4, 1024], fp8)
    fdt = inp("fdt", [B, 2, 1024])
    out = nc.dram_tensor("out", [B, 128, 2], f32, kind="ExternalOutput").ap()

    with tile.TileContext(nc) as tc:
        with (
            tc.tile_pool(name="const", bufs=1) as constp,
            tc.tile_pool(name="actp", bufs=3) as actp,
            tc.tile_pool(name="bigp", bufs=1) as bigp,
            tc.tile_pool(name="w7p", bufs=1) as w7p,
            tc.tile_pool(name="w8p", bufs=2) as w8p,
            tc.tile_pool(name="w9p", bufs=1) as w9p,
            tc.tile_pool(name="w10p", bufs=1) as w10p,
            tc.tile_pool(name="psA", bufs=2, space="PSUM") as psA,
            tc.tile_pool(name="psB", bufs=4, space="PSUM") as psB,
            tc.tile_pool(name="psC", bufs=2, space="PSUM") as psC,
            tc.tile_pool(name="dramp", bufs=1, space="DRAM") as dramp,
        ):
            # ---- resident loads, issued in consumption order
            r0sb = constp.tile([32, 64, 16], f32, name="r0sb")
            nc.sync.dma_start(r0sb[:], r0)
            w0sb = constp.tile([32, 128], f32, name="w0sb")
            nc.sync.dma_start(w0sb[:], w0)
            wpksb = constp.tile([128, 4, 128], f32, name="wpksb")
            nc.sync.dma_start(wpksb[:], wpk.rearrange("l p c -> p l c"))
            wmidsb = w7p.tile([128, 10752], bf16, name="wmidsb")
            # split so l5 can start before w6/w7 land
            nc.sync.dma_start(wmidsb[:, 0:512], wmid[:, 0:512])
            nc.sync.dma_start(wmidsb[:, 512:2560], wmid[:, 512:2560])
            nc.sync.dma_start(wmidsb[:, 2560:6656], wmid[:, 2560:6656])
            nc.sync.dma_start(wmidsb[:, 6656:10752], wmid[:, 6656:10752])
            w5sb = wmidsb[:, 0:512].rearrange("p (t c) -> p t c", c=256)
            w6sb = wmidsb[:, 512:2560].rearrange("p (t c) -> p t c", c=512)
            w7sb = wmidsb[:, 2560:10752].rearrange("p (t c) -> p t c", c=1024)

            # w9/w10 fully resident; slice DMAs let consumers start per-slice.
            # w10's dma_starts are issued AFTER the l9 matmuls in program
            # order so the sync engine queues them behind w9 (the x9 critical
            # path) instead of competing with it for HBM bandwidth.
            w9sb = w9p.tile([128, 8, 4, 512], bf16, name="w9sb")
            w10sb = w10p.tile([128, 16, 4, 1024], fp8, name="w10sb")

            # ---- input conv + packed levels 1..4 (all [128, 64, 16])
            xprev = None
            for lvl in range(5):
                # x4 feeds the bf16 level-5 matmul, so cast at the relu
                xn = actp.tile([128, 64, 16], bf16 if lvl == 4 else f32,
                               name=f"x{lvl}", tag="xl")
                for ch in range(2):
                    ps = psA.tile([128, 32, 16], f32, name="psA", tag="psA")
                    if lvl == 0:
                        nc.tensor.matmul(
                            ps[:], w0sb[:], r0sb[:, ch * 32:(ch + 1) * 32, :],
                            start=True, stop=True)
                    else:
                        nc.tensor.matmul(
                            ps[:], wpksb[:, lvl - 1, :],
                            xprev[:, ch * 32:(ch + 1) * 32, :],
                            start=True, stop=True)
                    nc.scalar.activation(
                        xn[:, ch * 32:(ch + 1) * 32, :], ps[:], RELU)
                xprev = xn

            # ---- standard levels (orientation A, weights stationary)
            def std_level(xin, wsb, cin_t, cout_t, w_out, name, out_tile=None):
                # xin [128, cin_t, 2*w_out, 16]; wsb [128, 2*cin_t, co] with
                # kt = k*cin_t + cit; returns [128, cout_t, w_out, 16]
                if out_tile is None:
                    xn = actp.tile([128, cout_t, w_out, 16], bf16,
                                   name=name, tag="xl")
                else:
                    xn = out_tile
                for ct in range(cout_t):
                    ps = psA.tile([128, w_out, 16], f32, name="psA", tag="psA")
                    for cit in range(cin_t):
                        rhs2 = xin[:, cit].rearrange(
                            "p (w two) b -> p two w b", two=2)
                        for k in range(2):
                            nc.tensor.matmul(
                                ps[:],
                                wsb[:, k * cin_t + cit,
                                    ct * 128:(ct + 1) * 128],
                                rhs2[:, k],
                                start=(cit == 0 and k == 0),
                                stop=(cit == cin_t - 1 and k == 1))
                    nc.scalar.activation(xn[:, ct], ps[:], RELU)
                return xn

            x5 = std_level(xprev[:, None], w5sb, 1, 2, 32, "x5")
            x6 = std_level(x5, w6sb, 2, 4, 16, "x6")
            x7 = std_level(x6, w7sb, 4, 8, 8, "x7")

            # ---- level 8 REPLICATED (full 2048 cout), co-major weight stream
            x8sb = bigp.tile([128, 16, 4, 16], bf16, name="x8sb")
            w8cs = []
            for c in range(4):
                w8c = w8p.tile([128, 16, 512], bf16, name="w8c", tag="w8c")
                nc.sync.dma_start(w8c[:], w8[c])
                w8cs.append(w8c)
            # w9 slice loads queue right behind w8 in FIFO order
            for m in range(8):
                nc.sync.dma_start(w9sb[:, m], w9[m])

            for c in range(4):
                w8c = w8cs[c]
                for ctl in range(4):
                    ps = psA.tile([128, 4, 16], f32, name="psA", tag="psA")
                    for cit in range(8):
                        rhs2 = x7[:, cit].rearrange(
                            "p (w two) b -> p two w b", two=2)
                        for k in range(2):
                            nc.tensor.matmul(
                                ps[:],
                                w8c[:, k * 8 + cit, ctl * 128:(ctl + 1) * 128],
                                rhs2[:, k],
                                start=(cit == 0 and k == 0),
                                stop=(cit == 7 and k == 1))
                    nc.scalar.activation(x8sb[:, c * 4 + ctl], ps[:], RELU)

            # ---- level 9 (512-ch shard, resident weights, 4 accumulators)
            ps9 = [psB.tile([128, 2, 16], f32, name=f"ps9_{ct}", tag="psB")
                   for ct in range(4)]
            for m in range(8):
                k, q = divmod(m, 4)
                for j in range(4):
                    cit = q * 4 + j
                    rhs = x8sb[:, cit].rearrange(
                        "p (w two) b -> p two w b", two=2)[:, k]
                    for ct in range(4):
                        nc.tensor.matmul(
                            ps9[ct][:],
                            w9sb[:, m, j, ct * 128:(ct + 1) * 128],
                            rhs,
                            start=(m == 0 and j == 0),
                            stop=(m == 7 and j == 3))
            x9loc = bigp.tile([128, 4, 2, 16], bf16, name="x9loc")
            for ct in range(4):
                nc.scalar.activation(x9loc[:, ct], ps9[ct][:], RELU)

            # w10 stream + fdsb queue behind everything the x9 path needs
            for m in range(16):
                nc.sync.dma_start(w10sb[:, m], w10[m])
            fdsb = constp.tile([B, 2, 1024], f32, name="fdsb")
            nc.sync.dma_start(fdsb[:], fdt)

            # ---- AllGather x9 -> full [128, 32, 2, 16]
            ag9_in = dramp.tile([1, 128, 4, 2, 16], bf16, name="ag9_in")
            ag9_out = dramp.tile([NCORES, 128, 4, 2, 16], bf16, name="ag9_out",
                                 addr_space="Shared")
            nc.sync.dma_start(ag9_in[0], x9loc[:])
            nc.gpsimd.collective_compute(
                "AllGather", mybir.AluOpType.bypass,
                replica_groups=[list(range(NCORES))],
                ins=[ag9_in.opt()], outs=[ag9_out.opt()])
            x9sb = bigp.tile([128, 32, 2, 16], bf16, name="x9sb")
            for r in range(NCORES):
                nc.sync.dma_start(x9sb[:, 4 * r:4 * r + 4], ag9_out[r])

            # ---- level 10 (1024-ch shard, orientation B: acts stationary,
            #      fp8 weights moving, resident in SBUF)
            ps10 = [psC.tile([B, 512], f32, name=f"ps10_{cb}", tag="psC")
                    for cb in range(2)]
            for m in range(16):
                k, q = divmod(m, 8)
                for j in range(4):
                    t = q * 4 + j
                    lhsT = x9sb[:, t, k, :]
                    for cb in range(2):
                        nc.tensor.matmul(
                            ps10[cb][:], lhsT,
                            w10sb[:, m, j, cb * 512:(cb + 1) * 512],
                            start=(m == 0 and j == 0),
                            stop=(m == 15 and j == 3))
            x10 = bigp.tile([B, 1024], f32, name="x10")
            for cb in range(2):
                nc.scalar.activation(
                    x10[:, cb * 512:(cb + 1) * 512], ps10[cb][:], RELU)

            # ---- final per-block einsum on the vector engine
            osb = bigp.tile([B, 128, 2], f32, name="osb")
            for o in range(2):
                prod = bigp.tile([B, 1024], f32, name=f"prod{o}")
                nc.vector.tensor_tensor(
                    prod[:], x10[:], fdsb[:, o, :], mybir.AluOpType.mult)
                nc.vector.tensor_reduce(
                    osb[:, :, o],
                    prod.rearrange("p (k c) -> p k c", c=8),
                    mybir.AxisListType.X, mybir.AluOpType.add)
            nc.sync.dma_start(out, osb[:])

    nc.compile()
    return nc


# ------------------------------------------------------------------- kernel

def kernel(**inputs):
    from concourse.bass_utils import run_bass_kernel_spmd

    in_maps = _host_prep(inputs)
    if "nc" not in _CACHE:
        _CACHE["nc"] = _build()
    nc = _CACHE["nc"]
    res = run_bass_kernel_spmd(nc, in_maps, core_ids=list(range(NCORES)))
    parts = [res.results[r]["out"] for r in range(NCORES)]  # each [16, 128, 2]
    full = np.concatenate(parts, axis=1)                    # [16, 1024, 2]
    return np.ascontiguousarray(full.reshape(B, 2048, 1).astype(np.float32))


# revision 6
# speedup vs baseline: 1.1449x; 1.1449x over previous
"""Trainium2 Bass kernel for the butterfly-CNN problem (nn_CNNLayer_30296699306356).

Network (see problem reference): input conv (k=2,s=2, 1->8 ch) + 10 butterfly
conv levels (k=2,s=2, channels double each level, relu, zero biases) + a
per-block dense matmul (1024 blocks of [8,2]) at the end.

Strategy (memory-regime; weights are ~358 MB fp32 dominated by levels 8-10):
  - Levels 5..9 run in bf16 (weights + activations, fp32 PSUM accumulation).
    Level 10 weights are float8e3 (e3m4) with per-output-channel scales that
    are folded into fea_dense on the host (relu commutes with positive
    scales), halving the dominant weight stream. Measured rel err ~1.4e-2
    (gate 2e-2, deterministic inputs).
  - Levels in..8 are replicated on all 8 cores; levels 9/10 shard the OUTPUT
    channels (1/8 of the dominant weight traffic per core). w9/w10 are fully
    RESIDENT in SBUF so the weight stream runs as one continuous FIFO.
  - x9 reassembly uses a hand-rolled one-shot all-to-all via
    remote_dma_broadcast (SBUF -> peer SBUF, ~5 us) instead of the gpsimd
    AllGather collective (~50 us of barrier+mesh latency). SPMD slot layout
    is XOR-based: slot j on core r holds core (r XOR j)'s x9 shard, which
    keeps every AP core-id-independent; the host permutes each core's w10
    chunk order to match. D2D engines deliver to tpb (requested ^ 2), so
    cross-die dests are requested pre-swapped (validated by probe.py).
  - Level 10 runs "orientation B" (x9 stationary, fp8 weights moving) with
    4-way PE column tiling: four independent 256-col matmul streams at array
    column offsets 0/32/64/96 accumulate into disjoint PSUM partition groups.
  - Final block einsum on the Vector engine across the 4 partition groups.

kernel(**inputs) takes the FULL unsharded inputs and returns the FULL output.
"""

import ml_dtypes
import numpy as np

NCORES = 8
B = 16
P = 128
C = 8
NLVL = 10
BF16 = ml_dtypes.bfloat16
FP8 = ml_dtypes.float8_e3m4
FP8_SCALE_TARGET = 14.0

_CACHE = {}


# ---------------------------------------------------------------- host prep

def _host_prep(inputs):
    """Build the per-core input maps (numpy only)."""
    ind = np.ascontiguousarray(np.asarray(inputs["in_data"], np.float32))
    f = {l: np.asarray(inputs[f"f{l}"], np.float32) for l in range(1, NLVL + 1)}
    f0 = np.asarray(inputs["in_filter"], np.float32)     # [2, 1, 8]
    fd = np.asarray(inputs["fea_dense"], np.float32)     # [1024, 8, 2]

    shared = {}
    # r0 [32, 64, 16]: r0[row, wHi, b] = in[b, wHi*32 + row]
    shared["r0"] = np.ascontiguousarray(
        ind[:, :, 0].reshape(B, 64, 32).transpose(2, 1, 0))

    # w0 [32, 128]: rows (2*wsub + k), cols (wsub*8 + co)
    w0 = np.zeros((32, 128), np.float32)
    for wsub in range(16):
        for k in range(2):
            w0[2 * wsub + k, wsub * 8:wsub * 8 + 8] = f0[k, 0, :]
    shared["w0"] = w0

    # packed levels 1..4 stacked: wpk [4, 128, 128]
    wpk = np.zeros((4, 128, 128), np.float32)
    for lvl in range(1, 5):
        cin = 2 ** (lvl - 1) * C
        cout = 2 ** lvl * C
        s_out = (128 // cin) // 2
        for wso in range(s_out):
            for k in range(2):
                wsi = 2 * wso + k
                wpk[lvl - 1, wsi * cin:(wsi + 1) * cin,
                    wso * cout:(wso + 1) * cout] = f[lvl][k]
    shared["wpk"] = wpk

    # w5/w6/w7 mega-packed [128, 10752] bf16 (kt-major per level), one tile
    w5v = f[5].astype(BF16).reshape(2, 1, 128, 256)
    w6v = f[6].astype(BF16).reshape(2, 2, 128, 512)
    w7v = f[7].astype(BF16).reshape(2, 4, 128, 1024)
    shared["wmid"] = np.ascontiguousarray(np.concatenate([
        w5v.transpose(2, 0, 1, 3).reshape(128, 512),
        w6v.transpose(2, 0, 1, 3).reshape(128, 2048),
        w7v.transpose(2, 0, 1, 3).reshape(128, 8192)], axis=1))

    # f8 is REPLICATED: co-major chunks [4, 128, kt=16, co=512], kt = k*8 + cit
    f8b = f[8].astype(BF16)
    w8full = np.stack([
        np.ascontiguousarray(
            f8b[:, :, c * 512:(c + 1) * 512]
            .reshape(2, 8, 128, 512).transpose(2, 0, 1, 3).reshape(128, 16, 512))
        for c in range(4)])
    shared["w8"] = w8full

    # f9 output-channel shards, packed into 4-ci-tile chunks:
    # [8, 128, 4, 512]; chunk m = k*4 + q, cit = q*4+j
    w9s = []
    f9b = f[9].astype(BF16)
    for r in range(NCORES):
        blk = f9b[:, :, r * 512:(r + 1) * 512]
        v = blk.reshape(2, 4, 4, 128, 512).transpose(0, 1, 3, 2, 4)
        w9s.append(np.ascontiguousarray(v.reshape(8, 128, 4, 512)))

    # f10 output-channel shards in float8_e3m4 with per-output-channel scales
    # (folded into fea_dense below): [16, 128, 4, 1024] fp8.
    # Chunk m = k*8 + j where j is the XOR exchange SLOT: the input-channel
    # block is q = r ^ j (slot j of the gathered x9 holds core (r^j)'s shard).
    s10 = np.max(np.abs(f[10]), axis=(0, 1)) / FP8_SCALE_TARGET  # [8192]
    f10q = (f[10] / s10[None, None, :]).astype(FP8)
    w10s = []
    for r in range(NCORES):
        v = f10q[:, :, r * 1024:(r + 1) * 1024].reshape(2, 8, 4, 128, 1024)
        chunks = []
        for m in range(16):
            k, j = divmod(m, 8)
            q = r ^ j
            chunks.append(v[k, q].transpose(1, 0, 2))     # [128, 4, 1024]
        w10s.append(np.ascontiguousarray(np.stack(chunks)))

    # fea_dense shard with the fp8 scales folded in, packed for the 4 PE
    # column groups: fdt[32*g + b, o, c] = fd_flat[o, g*256 + c] * s10[...]
    fds = []
    for r in range(NCORES):
        blk = fd[r * 128:(r + 1) * 128]                    # [128, 8, 2]
        flat = blk.transpose(2, 0, 1).reshape(2, 1024)     # [o, 1024]
        flat = flat * s10[r * 1024:(r + 1) * 1024][None, :]
        ft = np.zeros((128, 2, 256), np.float32)
        for g in range(4):
            ft[32 * g:32 * g + B] = np.broadcast_to(
                flat[None, :, 256 * g:256 * (g + 1)], (B, 2, 256))
        fds.append(np.ascontiguousarray(ft))

    in_maps = []
    for r in range(NCORES):
        m = dict(shared)
        m["w9"] = w9s[r]
        m["w10"] = w10s[r]
        m["fdt"] = fds[r]
        in_maps.append(m)
    return in_maps


# ---------------------------------------------------------------- bass build

def _build():
    import concourse.bass as bass
    import concourse.mybir as mybir
    import concourse.tile as tile
    from concourse import bacc

    f32 = mybir.dt.float32
    bf16 = mybir.dt.bfloat16
    fp8 = mybir.dt.float8e3
    RELU = mybir.ActivationFunctionType.Relu

    nc = bacc.Bacc("TRN2", target_bir_lowering=False, debug=False,
                   num_devices=NCORES)

    def inp(name, shape, dt=f32):
        return nc.dram_tensor(name, shape, dt, kind="ExternalInput").ap()

    r0 = inp("r0", [32, 64, 16])
    w0 = inp("w0", [32, 128])
    wpk = inp("wpk", [4, 128, 128])
    wmid = inp("wmid", [128, 10752], bf16)
    w8 = inp("w8", [4, 128, 16, 512], bf16)
    w9 = inp("w9", [8, 128, 4, 512], bf16)
    w10 = inp("w10", [16, 128, 4, 1024], fp8)
    fdt = inp("fdt", [128, 2, 256])
    out = nc.dram_tensor("out", [B, 128, 2], f32, kind="ExternalOutput").ap()

    xsem = nc.alloc_semaphore("x9_xsem")
    lsem = nc.alloc_semaphore("x9_lsem")
    psem = nc.alloc_semaphore("x9_psem")

    with tile.TileContext(nc) as tc:
        with (
            tc.tile_pool(name="const", bufs=1) as constp,
            tc.tile_pool(name="actp", bufs=3) as actp,
            tc.tile_pool(name="bigp", bufs=1) as bigp,
            tc.tile_pool(name="w7p", bufs=1) as w7p,
            tc.tile_pool(name="w8p", bufs=3) as w8p,
            tc.tile_pool(name="w9p", bufs=1) as w9p,
            tc.tile_pool(name="w10p", bufs=1) as w10p,
            tc.tile_pool(name="psA", bufs=2, space="PSUM") as psA,
            tc.tile_pool(name="psB", bufs=4, space="PSUM") as psB,
            tc.tile_pool(name="psC", bufs=1, space="PSUM") as psC,
        ):
            # ---- resident loads, issued in consumption order
            r0sb = constp.tile([32, 64, 16], f32, name="r0sb")
            nc.sync.dma_start(r0sb[:], r0)
            w0sb = constp.tile([32, 128], f32, name="w0sb")
            nc.sync.dma_start(w0sb[:], w0)
            wpksb = constp.tile([128, 4, 128], f32, name="wpksb")
            nc.sync.dma_start(wpksb[:], wpk.rearrange("l p c -> p l c"))
            wmidsb = w7p.tile([128, 10752], bf16, name="wmidsb")
            # split so l5 can start before w6/w7 land
            nc.sync.dma_start(wmidsb[:, 0:512], wmid[:, 0:512])
            nc.sync.dma_start(wmidsb[:, 512:2560], wmid[:, 512:2560])
            nc.sync.dma_start(wmidsb[:, 2560:6656], wmid[:, 2560:6656])
            nc.sync.dma_start(wmidsb[:, 6656:10752], wmid[:, 6656:10752])
            w5sb = wmidsb[:, 0:512].rearrange("p (t c) -> p t c", c=256)
            w6sb = wmidsb[:, 512:2560].rearrange("p (t c) -> p t c", c=512)
            w7sb = wmidsb[:, 2560:10752].rearrange("p (t c) -> p t c", c=1024)

            # w9/w10 fully resident; slice DMAs let consumers start per-slice
            w9sb = w9p.tile([128, 8, 4, 512], bf16, name="w9sb")
            w10sb = w10p.tile([128, 16, 4, 1024], fp8, name="w10sb")

            # ---- input conv + packed levels 1..4 (all [128, 64, 16])
            xprev = None
            for lvl in range(5):
                # x4 feeds the bf16 level-5 matmul, so cast at the relu
                xn = actp.tile([128, 64, 16], bf16 if lvl == 4 else f32,
                               name=f"x{lvl}", tag="xl")
                for ch in range(2):
                    ps = psA.tile([128, 32, 16], f32, name="psA", tag="psA")
                    if lvl == 0:
                        nc.tensor.matmul(
                            ps[:], w0sb[:], r0sb[:, ch * 32:(ch + 1) * 32, :],
                            start=True, stop=True)
                    else:
                        nc.tensor.matmul(
                            ps[:], wpksb[:, lvl - 1, :],
                            xprev[:, ch * 32:(ch + 1) * 32, :],
                            start=True, stop=True)
                    nc.scalar.activation(
                        xn[:, ch * 32:(ch + 1) * 32, :], ps[:], RELU)
                xprev = xn

            # ---- standard levels (orientation A, weights stationary)
            def std_level(xin, wsb, cin_t, cout_t, w_out, name, out_tile=None):
                # xin [128, cin_t, 2*w_out, 16]; wsb [128, 2*cin_t, co] with
                # kt = k*cin_t + cit; returns [128, cout_t, w_out, 16]
                if out_tile is None:
                    xn = actp.tile([128, cout_t, w_out, 16], bf16,
                                   name=name, tag="xl")
                else:
                    xn = out_tile
                for ct in range(cout_t):
                    ps = psA.tile([128, w_out, 16], f32, name="psA", tag="psA")
                    for cit in range(cin_t):
                        rhs2 = xin[:, cit].rearrange(
                            "p (w two) b -> p two w b", two=2)
                        for k in range(2):
                            nc.tensor.matmul(
                                ps[:],
                                wsb[:, k * cin_t + cit,
                                    ct * 128:(ct + 1) * 128],
                                rhs2[:, k],
                                start=(cit == 0 and k == 0),
                                stop=(cit == cin_t - 1 and k == 1))
                    nc.scalar.activation(xn[:, ct], ps[:], RELU)
                return xn

            x5 = std_level(xprev[:, None], w5sb, 1, 2, 32, "x5")
            x6 = std_level(x5, w6sb, 2, 4, 16, "x6")
            x7 = std_level(x6, w7sb, 4, 8, 8, "x7")

            # ---- level 8 REPLICATED (full 2048 cout), co-major weight stream
            x8sb = bigp.tile([128, 16, 4, 16], bf16, name="x8sb")
            w8cs = []
            for c in range(4):
                w8c = w8p.tile([128, 16, 512], bf16, name="w8c", tag="w8c")
                nc.sync.dma_start(w8c[:], w8[c])
                w8cs.append(w8c)
            # w9/w10/fdt descriptors queue behind w8 on the sync engine
            for m in range(8):
                nc.sync.dma_start(w9sb[:, m], w9[m])
            for m in range(16):
                nc.sync.dma_start(w10sb[:, m], w10[m])
            fdsb = constp.tile([128, 2, 256], f32, name="fdsb")
            nc.sync.dma_start(fdsb[:], fdt)

            for c in range(4):
                w8c = w8cs[c]
                for ctl in range(4):
                    ps = psA.tile([128, 4, 16], f32, name="psA", tag="psA")
                    for cit in range(8):
                        rhs2 = x7[:, cit].rearrange(
                            "p (w two) b -> p two w b", two=2)
                        for k in range(2):
                            nc.tensor.matmul(
                                ps[:],
                                w8c[:, k * 8 + cit, ctl * 128:(ctl + 1) * 128],
                                rhs2[:, k],
                                start=(cit == 0 and k == 0),
                                stop=(cit == 7 and k == 1))
                    nc.scalar.activation(x8sb[:, c * 4 + ctl], ps[:], RELU)

            # ---- level 9 (512-ch shard, resident weights, 4 accumulators)
            ps9 = [psB.tile([128, 2, 16], f32, name=f"ps9_{ct}", tag="psB")
                   for ct in range(4)]
            for m in range(8):
                k, q = divmod(m, 4)
                for j in range(4):
                    cit = q * 4 + j
                    rhs = x8sb[:, cit].rearrange(
                        "p (w two) b -> p two w b", two=2)[:, k]
                    for ct in range(4):
                        nc.tensor.matmul(
                            ps9[ct][:],
                            w9sb[:, m, j, ct * 128:(ct + 1) * 128],
                            rhs,
                            start=(m == 0 and j == 0),
                            stop=(m == 7 and j == 3))

            # ---- x9 all-to-all exchange (XOR slots): x9x[:, j] holds core
            # (r^j)'s [128, 4, 2, 16] shard; slot 0 is written locally.
            x9x = bigp.tile([128, 8, 4, 2, 16], bf16, name="x9x")
            for ct in range(4):
                nc.scalar.activation(x9x[:, 0, ct], ps9[ct][:], RELU)

            with tc.tile_critical():
                nc.gpsimd.bir_kernel_barrier_wait([list(range(NCORES))])
                nc.gpsimd.sem_clear(psem)
                for i in range(1, NCORES):
                    rd = [None] * 8
                    # D2D engines deliver to tpb (requested ^ 2): pre-swap
                    v = i ^ 2 if i & 4 else i
                    rd[v] = (0, v)
                    nc.gpsimd.remote_dma_broadcast(
                        x9x[:, i], x9x[:, 0],
                        remote_sem=xsem, local_sem=lsem, rdests=rd
                    ).then_inc(psem, 1)
                nc.gpsimd.wait_ge(psem, 7)
                nc.gpsimd.trigger_dma(count=7)

            x9sb = bigp.tile([128, 8, 4, 2, 16], bf16, name="x9sb")
            with tc.tile_critical():
                nc.vector.wait_ge(xsem, 14)
                nc.vector.sem_clear(xsem)
                nc.vector.tensor_scalar_add(x9sb[:], x9x[:], 0.0)

            # ---- level 10 (1024-ch shard, orientation B, fp8 weights moving,
            #      4-way PE column tiling: group g -> array cols 32g, PSUM
            #      partitions [32g, 32g+16), output cols [256g, 256(g+1)))
            ps10 = psC.tile([128, 256], f32, name="ps10")
            for m in range(16):
                k, j = divmod(m, 8)
                xsrc = x9x if j == 0 else x9sb
                for jj in range(4):
                    lhsT = xsrc[:, j, jj, k, :]
                    for g in range(4):
                        nc.tensor.matmul(
                            ps10[32 * g:32 * g + B, :], lhsT,
                            w10sb[:, m, jj, 256 * g:256 * (g + 1)],
                            start=(m == 0 and jj == 0),
                            stop=(m == 15 and jj == 3),
                            tile_position=(0, 32 * g),
                            skip_group_check=True)
            x10 = bigp.tile([128, 256], f32, name="x10")
            for g in range(4):
                nc.scalar.activation(
                    x10[32 * g:32 * g + B, :], ps10[32 * g:32 * g + B, :],
                    RELU)

            # ---- final per-block einsum on the vector engine
            osb = bigp.tile([128, 32, 2], f32, name="osb")
            for o in range(2):
                prod = bigp.tile([128, 256], f32, name=f"prod{o}")
                nc.vector.tensor_tensor(
                    prod[:], x10[:], fdsb[:, o, :], mybir.AluOpType.mult)
                nc.vector.tensor_reduce(
                    osb[:, :, o],
                    prod.rearrange("p (k c) -> p k c", c=8),
                    mybir.AxisListType.X, mybir.AluOpType.add)
            for g in range(4):
                nc.sync.dma_start(out[:, 32 * g:32 * (g + 1), :],
                                  osb[32 * g:32 * g + B, :, :])

    nc.compile()
    return nc


# ------------------------------------------------------------------- kernel

def kernel(**inputs):
    from concourse.bass_utils import run_bass_kernel_spmd

    in_maps = _host_prep(inputs)
    if "nc" not in _CACHE:
        _CACHE["nc"] = _build()
    nc = _CACHE["nc"]
    res = run_bass_kernel_spmd(nc, in_maps, core_ids=list(range(NCORES)))
    parts = [res.results[r]["out"] for r in range(NCORES)]  # each [16, 128, 2]
    full = np.concatenate(parts, axis=1)                    # [16, 1024, 2]
    return np.ascontiguousarray(full.reshape(B, 2048, 1).astype(np.float32))
